# revision 21
# baseline (speedup 1.0000x reference)
"""Trainium2 Bass kernel for nn_MultiHeadAttention (B=4, N=2048, C=512, H=8).

Returns (out [B,N,C] f32, attn_mean [B,N,N] f32) like the reference.

Sharding: 8 cores = 4 batches x 2 query-halves, no collectives. The
query-half selection is done by rolling x/mask on the host so the SPMD
program is identical on every core; attn_mean columns are rolled back on
the host.

Per-core dataflow (matmuls bf16, PSUM accumulation f32):
  setup: x^T via PE transpose (bf16); per-head Q~^T [65, nq] (0.125-scaled
         rows + ones row) and K~^T [65, nfull] (rows + maskbias row of
         -1e30 on masked keys); V [k, C] (+bias via ones-row matmul).
  per head, per q-tile: S = Q~^T.T @ K~^T (mask folded in) -> one exp pass
         (ACT accum_out gives the softmax denominator Z for free) ->
         P-hat = P~ * (1/Z)  (DVE 4x) -> attn_mean accumulates P-hat (DVE
         2x adds); P-hat is DMA-xbar-transposed into k-major slabs feeding
         O^T = V.T @ P-hat^T (PE, comes out pre-normalized), and O^T rows
         feed the output projection directly (no PE transposes of O).
"""

import numpy as np
import ml_dtypes

B, NFULL, C = 4, 2048, 512
H, D = 8, 64
NQ = NFULL // 2  # query rows per core
N_CORES = 8
SCALE = D ** -0.5

_prog_cache = {}


def _pin_act_tables():
    import concourse.bacc as bacc_mod
    import concourse.mybir as mybir
    if getattr(bacc_mod, "_act_tables_pinned", False):
        return
    orig = bacc_mod.get_activation_tables

    def pinned(arch):
        t = dict(orig(arch))
        keep = "natural_log_exp_and_others"
        drop = {mybir.ActivationFunctionType.Exp,
                mybir.ActivationFunctionType.Ln}
        if keep in t:
            for name in t:
                if name != keep:
                    t[name] = t[name] - drop
        return t

    bacc_mod.get_activation_tables = pinned
    bacc_mod._act_tables_pinned = True


def _build_program(nfull=NFULL, nq=NQ):
    import concourse.bass as bass
    import concourse.tile as tile
    import concourse.mybir as mybir
    from concourse import bacc
    from concourse.masks import make_identity
    _pin_act_tables()

    dt = mybir.dt
    FP32 = dt.float32
    BF16 = dt.bfloat16
    AF = mybir.ActivationFunctionType
    OP = mybir.AluOpType

    NT = nfull // 128      # k/v row tiles
    CT = C // 128          # channel tiles (4)
    QT = nq // 128         # query tiles per core
    KH = max(1, nfull // 1024)   # exp psum chunks (1024 wide)
    KHW = min(1024, nfull)
    Q5 = max(1, nq // 512)       # 512-wide chunks of q
    W5 = min(512, nq)
    NG = 2 if QT >= 2 else 1     # slab groups
    GQT = QT // NG               # q-tiles per group
    GW = GQT * 128               # group width in q

    nc = bacc.Bacc("TRN2", target_bir_lowering=False, debug=False,
                   num_devices=N_CORES)

    xb = nc.dram_tensor("xb", [nfull, C], FP32, kind="ExternalInput").ap()
    maskv = nc.dram_tensor("maskv", [nfull], dt.int32, kind="ExternalInput").ap()
    wqkv = nc.dram_tensor("wqkv", [C, 3 * C], FP32, kind="ExternalInput").ap()
    bqkv = nc.dram_tensor("bqkv", [3 * C], FP32, kind="ExternalInput").ap()
    wproj = nc.dram_tensor("wproj", [C, C], FP32, kind="ExternalInput").ap()
    bproj = nc.dram_tensor("bproj", [C], FP32, kind="ExternalInput").ap()
    out_o = nc.dram_tensor("out_o", [nq, C], FP32, kind="ExternalOutput").ap()
    out_a = nc.dram_tensor("out_a", [nq, nfull], BF16, kind="ExternalOutput").ap()

    with tile.TileContext(nc) as tc:
        from contextlib import ExitStack
        ctx = ExitStack()
        with ctx:
            pers = ctx.enter_context(tc.tile_pool(name="pers", bufs=1))

            # PSUM: S-stream [128,1024] x3 (6 banks) + small [.,<=512] x2
            sp = ctx.enter_context(tc.tile_pool(name="sp", bufs=3, space="PSUM"))
            ovp = ctx.enter_context(tc.tile_pool(name="ovp", bufs=2, space="PSUM"))

            id16 = pers.tile([128, 128], BF16, tag="id16")
            make_identity(nc, id16[:])

            bpb = pers.tile([128, C], FP32, tag="bpb")      # bproj broadcast
            onesrow = pers.tile([1, nfull], BF16, tag="onesrow")
            nc.vector.memset(onesrow[:], 1.0)

            QTM = [pers.tile([D + 1, nq], BF16, tag="qtm", bufs=H,
                             name=f"qtm{i}") for i in range(H)]
            KTM = [pers.tile([D + 1, nfull], BF16, tag="ktm", bufs=H,
                             name=f"ktm{i}") for i in range(H)]
            Vt = pers.tile([128, NT, C], BF16, tag="vt")
            WPt = [pers.tile([D, C], BF16, tag="wp", bufs=H, name=f"wp{i}")
                   for i in range(H)]

            # ---------- setup (transient pools) ----------
            with tc.tile_pool(name="stage", bufs=2) as stg, \
                 tc.tile_pool(name="sxt", bufs=1) as sxt, \
                 tc.tile_pool(name="svec", bufs=1) as svec:

                # mask -> f32 row -> maskbias row (-1e30 on masked keys)
                mri = svec.tile([1, nfull], dt.int32, tag="mri")
                nc.sync.dma_start(mri[:], maskv.unsqueeze(0))
                mrf = svec.tile([1, nfull], FP32, tag="mrf")
                nc.vector.tensor_copy(mrf[:], mri[:])
                mbias = svec.tile([1, nfull], BF16, tag="mbias")
                nc.vector.tensor_scalar(mbias[:], mrf[:], 1e30, -1e30,
                                        op0=OP.mult, op1=OP.add)
                for h in range(H):
                    nc.sync.dma_start(KTM[h][D:D + 1, :], mbias[:])
                    nc.sync.dma_start(QTM[h][D:D + 1, :], onesrow[0:1, 0:nq])

                # b_proj broadcast via ones-column matmul
                bprow = svec.tile([1, C], FP32, tag="bprow")
                nc.sync.dma_start(bprow[:], bproj.unsqueeze(0))
                ones32 = svec.tile([1, 128], FP32, tag="ones32")
                nc.vector.memset(ones32[:], 1.0)
                bb2 = ovp.tile([128, C], FP32, tag="pv", name="bb2")
                nc.tensor.matmul(bb2[:], ones32[:], bprow[:])
                nc.vector.tensor_copy(bpb[:], bb2[:])

                # qkv bias vectors [128, 12] (column t = b[128t:128(t+1)])
                bq12 = svec.tile([128, 3 * CT], FP32, tag="bq12")
                nc.sync.dma_start(bq12[:], bqkv.rearrange("(t p) -> p t", p=128))
                bqs = svec.tile([128, CT], FP32, tag="bqs")  # 0.125 * b_q
                nc.vector.tensor_scalar(bqs[:], bq12[:, 0:CT], SCALE, None,
                                        op0=OP.mult)

                # x -> SBUF bf16, PE-transpose -> xT
                xT = [sxt.tile([128, nfull], BF16, tag="xT", bufs=CT,
                               name=f"xT{i}") for i in range(CT)]
                for nt in range(NT):
                    xs = stg.tile([128, C], FP32, tag="xs")
                    nc.sync.dma_start(xs[:], xb[nt * 128:(nt + 1) * 128, :])
                    xsb = stg.tile([128, C], BF16, tag="xsb")
                    nc.vector.tensor_copy(xsb[:], xs[:])
                    for ct in range(CT):
                        tp = ovp.tile([128, 128], BF16, tag="pv", name="tp")
                        nc.tensor.transpose(tp[:],
                                            xsb[:, ct * 128:(ct + 1) * 128],
                                            id16[:])
                        nc.vector.tensor_copy(
                            xT[ct][:, nt * 128:(nt + 1) * 128], tp[:])

                # W_qkv -> bf16 (Q columns pre-scaled by 0.125)
                Wc = [sxt.tile([128, 3 * C], BF16, tag="wc", bufs=CT,
                               name=f"wc{i}") for i in range(CT)]
                bvrow = sxt.tile([1, C], BF16, tag="bvrow")
                bvf = svec.tile([1, 3 * C], FP32, tag="bvf")
                nc.sync.dma_start(bvf[:], bqkv.unsqueeze(0))
                nc.vector.tensor_copy(bvrow[:], bvf[0:1, 2 * C:3 * C])
                for ct in range(CT):
                    ws = stg.tile([128, 3 * C], FP32, tag="ws")
                    nc.sync.dma_start(ws[:], wqkv[ct * 128:(ct + 1) * 128, :])
                    nc.vector.tensor_scalar(Wc[ct][:, 0:C], ws[:, 0:C], SCALE,
                                            None, op0=OP.mult)
                    nc.vector.tensor_copy(Wc[ct][:, C:3 * C], ws[:, C:3 * C])

                # W_proj -> per-head [64, C] bf16 rows (base partition 0)
                for h in range(H):
                    wps = stg.tile([D, C], FP32, tag="wps")
                    nc.sync.dma_start(wps[:], wproj[h * D:(h + 1) * D, :])
                    nc.vector.tensor_copy(WPt[h][:], wps[:])

                # Q^T d-pair tiles -> per-head QTM rows 0:64 (bias added)
                for j in range(CT):
                    ps = sp.tile([128, nq], FP32, tag="s", name="psq")
                    for ct in range(CT):
                        for q5 in range(Q5):
                            nc.tensor.matmul(
                                ps[:, q5 * W5:(q5 + 1) * W5],
                                Wc[ct][:, j * 128:(j + 1) * 128],
                                xT[ct][:, q5 * W5:(q5 + 1) * W5],
                                start=(ct == 0), stop=(ct == CT - 1))
                    for hh in range(2):
                        nc.vector.tensor_scalar(
                            QTM[2 * j + hh][0:D, :],
                            ps[hh * D:(hh + 1) * D, :],
                            bqs[hh * D:(hh + 1) * D, j:j + 1], None,
                            op0=OP.add)

                # K^T d-pair tiles -> per-head KTM rows 0:64 (bias added)
                for j in range(CT):
                    for kc in range(nfull // nq):
                        ps = sp.tile([128, nq], FP32, tag="s", name="psk")
                        for ct in range(CT):
                            for q5 in range(Q5):
                                o = kc * nq + q5 * W5
                                nc.tensor.matmul(
                                    ps[:, q5 * W5:(q5 + 1) * W5],
                                    Wc[ct][:, C + j * 128:C + (j + 1) * 128],
                                    xT[ct][:, o:o + W5],
                                    start=(ct == 0), stop=(ct == CT - 1))
                        for hh in range(2):
                            nc.vector.tensor_scalar(
                                KTM[2 * j + hh][0:D, kc * nq:(kc + 1) * nq],
                                ps[hh * D:(hh + 1) * D, :],
                                bq12[hh * D:(hh + 1) * D, CT + j:CT + j + 1],
                                None, op0=OP.add)

                # V tiles [128, NT, C] with bias (ones-row trick)
                for nt in range(NT):
                    ps = ovp.tile([128, C], FP32, tag="pv", name="psv")
                    for ct in range(CT):
                        nc.tensor.matmul(ps[:],
                                         xT[ct][:, nt * 128:(nt + 1) * 128],
                                         Wc[ct][:, 2 * C:3 * C],
                                         start=(ct == 0), stop=False)
                    nc.tensor.matmul(ps[:],
                                     onesrow[0:1, nt * 128:(nt + 1) * 128],
                                     bvrow[0:1, :], start=False, stop=True)
                    nc.vector.tensor_copy(Vt[:, nt, :], ps[:])

            # ---------- main-loop pools ----------
            pMAIN = ctx.enter_context(tc.tile_pool(name="pMAIN", bufs=1))
            pSLAB = ctx.enter_context(tc.tile_pool(name="pSLAB", bufs=2))
            pPH = ctx.enter_context(tc.tile_pool(name="pPH", bufs=3))
            pPP = ctx.enter_context(tc.tile_pool(name="pPP", bufs=3))
            pZ = ctx.enter_context(tc.tile_pool(name="pZ", bufs=4))
            pOUT = ctx.enter_context(tc.tile_pool(name="pOUT", bufs=2))

            At = [pMAIN.tile([128, nfull], BF16, tag="a", bufs=QT,
                             name=f"a{i}") for i in range(QT)]
            OTt = [pMAIN.tile([D, nq], BF16, tag="oth", bufs=H,
                              name=f"oth{i}") for i in range(H)]
            RZt = [pMAIN.tile([128, H], FP32, tag="rz", bufs=QT,
                              name=f"rz{i}") for i in range(QT)]

            # ---------- main loop ----------
            ov_state = {}

            def _emit_pv(h, g, slabs, kt0, nkt):
                w = min(512, GW)
                for g5 in range(GW // 512 if GW >= 512 else 1):
                    key = (g, g5)
                    if kt0 == 0:
                        ov_state[key] = ovp.tile([D, w], FP32, tag="pv",
                                                 name="ov")
                    ov = ov_state[key]
                    for kt in range(kt0, kt0 + nkt):
                        nc.tensor.matmul(
                            ov[:],
                            Vt[:, kt, h * D:(h + 1) * D],
                            slabs[g][:, kt, g5 * w:(g5 + 1) * w],
                            start=(kt == 0), stop=(kt == NT - 1))
                    if kt0 + nkt == NT:
                        nc.vector.tensor_copy(
                            OTt[h][:, g * GW + g5 * w:g * GW + (g5 + 1) * w],
                            ov[:])

            for h in range(H):
                slabs = [pSLAB.tile([128, NT, GW], BF16, tag="slab",
                                    name=f"slab{h}_{g}") for g in range(NG)]
                for qt in range(QT):
                    tl = []
                    for kh in range(KH):
                        ps = sp.tile([128, KHW], FP32, tag="s", name="ps")
                        for c5 in range(KHW // 512 if KHW >= 512 else 1):
                            w = min(512, KHW)
                            o = kh * KHW + c5 * w
                            nc.tensor.matmul(
                                ps[:, c5 * w:(c5 + 1) * w],
                                QTM[h][:, qt * 128:(qt + 1) * 128],
                                KTM[h][:, o:o + w])
                        tl.append(ps)
                    # one exp pass; accum_out gives row-sum partials
                    ph = pPH.tile([128, nfull], BF16, tag="ph")
                    zps = []
                    for kh in range(KH):
                        zp = pZ.tile([128, 1], FP32, tag="zp", name="zp")
                        pp = pPP.tile([128, KHW], BF16, tag="pp", name="pp")
                        nc.scalar.activation(pp[:], tl[kh][:], AF.Exp,
                                             accum_out=zp[:])
                        zps.append((zp, pp))
                    if KH == 2:
                        nc.vector.tensor_tensor(zps[0][0][:], zps[0][0][:],
                                                zps[1][0][:], op=OP.add)
                    nc.vector.reciprocal(RZt[qt][:, h:h + 1], zps[0][0][:])
                    # P-hat = P~ * rz  (normalized + masked)
                    for kh in range(KH):
                        nc.vector.tensor_scalar(
                            ph[:, kh * KHW:(kh + 1) * KHW], zps[kh][1][:],
                            RZt[qt][:, h:h + 1], None, op0=OP.mult)
                    # attn-mean accumulation
                    if h == 0:
                        nc.vector.tensor_copy(At[qt][:], ph[:])
                    else:
                        nc.vector.tensor_tensor(At[qt][:], ph[:], At[qt][:],
                                                op=OP.add)
                    # k-major transpose into the slab group
                    g = qt // GQT
                    nc.sync.dma_start_transpose(
                        slabs[g][:, :, (qt % GQT) * 128:(qt % GQT + 1) * 128],
                        ph[:])
                # O^T = V.T @ P-hat^T accumulated over k tiles
                for g in range(NG):
                    _emit_pv(h, g, slabs, 0, NT)

            # ---------- tail: DMA attn out; projection ----------
            for qt in range(QT):
                nc.gpsimd.dma_start(out_a[qt * 128:(qt + 1) * 128, :], At[qt][:])

            for qt in range(QT):
                pj = ovp.tile([128, C], FP32, tag="pv", name="pj")
                for h in range(H):
                    nc.tensor.matmul(pj[:],
                                     OTt[h][:, qt * 128:(qt + 1) * 128],
                                     WPt[h][:], start=(h == 0),
                                     stop=(h == H - 1))
                ob = pOUT.tile([128, C], FP32, tag="ob")
                nc.vector.tensor_tensor(ob[:], pj[:], bpb[:], op=OP.add)
                nc.gpsimd.dma_start(out_o[qt * 128:(qt + 1) * 128, :], ob[:])

    nc.compile()
    return nc


def _get_program(nfull=NFULL, nq=NQ):
    key = (nfull, nq)
    if key not in _prog_cache:
        _prog_cache[key] = _build_program(nfull, nq)
    return _prog_cache[key]


def _run(x, mask, W_qkv, b_qkv, W_proj, b_proj, nfull, nq):
    from concourse.bass_utils import run_bass_kernel_spmd

    nbatch = x.shape[0]
    halves = N_CORES // nbatch
    nc = _get_program(nfull, nq)
    in_maps = []
    for c in range(N_CORES):
        b, j = c // halves, c % halves
        in_maps.append({
            "xb": np.roll(x[b], -j * nq, axis=0) if j else x[b],
            "maskv": np.roll(mask[b], -j * nq) if j else mask[b],
            "wqkv": W_qkv, "bqkv": b_qkv, "wproj": W_proj, "bproj": b_proj,
        })
    res = run_bass_kernel_spmd(nc, in_maps, list(range(N_CORES)))

    out = np.empty((nbatch, nfull, C), np.float32)
    attn = np.empty((nbatch, nfull, nfull), np.float32)
    for c in range(N_CORES):
        b, j = c // halves, c % halves
        out[b, j * nq:(j + 1) * nq] = res.results[c]["out_o"]
        a = res.results[c]["out_a"].astype(np.float32) / H
        if j:
            a = np.roll(a, j * nq, axis=1)
        attn[b, j * nq:(j + 1) * nq] = a
    return out, attn


def kernel(x, mask, W_qkv, b_qkv, W_proj, b_proj):
    return _run(np.asarray(x, np.float32), np.asarray(mask, np.int32),
                np.asarray(W_qkv, np.float32), np.asarray(b_qkv, np.float32),
                np.asarray(W_proj, np.float32), np.asarray(b_proj, np.float32),
                NFULL, NQ)


# revision 22
# speedup vs baseline: 1.0554x; 1.0554x over previous
"""Trainium2 Bass kernel for nn_MultiHeadAttention (B=4, N=2048, C=512, H=8).

Returns (out [B,N,C] f32, attn_mean [B,N,N] f32) like the reference.

Sharding: 8 cores = 4 batches x 2 query-halves, no collectives. The
query-half selection is done by rolling x/mask on the host so the SPMD
program is identical on every core; attn_mean columns are rolled back on
the host.

Per-core dataflow (matmuls bf16, PSUM accumulation f32):
  setup: x^T via PE transpose (bf16); per-head Q~^T [65, nq] (0.125-scaled
         rows + ones row) and K~^T [65, nfull] (rows + maskbias row of
         -1e30 on masked keys); V [k, C] (+bias via ones-row matmul).
  per head, per q-tile: S = Q~^T.T @ K~^T (mask folded in) -> one exp pass
         (ACT accum_out gives the softmax denominator Z for free) ->
         P-hat = P~ * (1/Z)  (DVE 4x) -> attn_mean accumulates P-hat (DVE
         2x adds); P-hat is DMA-xbar-transposed into k-major slabs feeding
         O^T = V.T @ P-hat^T (PE, comes out pre-normalized), and O^T rows
         feed the output projection directly (no PE transposes of O).
"""

import numpy as np
import ml_dtypes

B, NFULL, C = 4, 2048, 512
H, D = 8, 64
NQ = NFULL // 2  # query rows per core
N_CORES = 8
SCALE = D ** -0.5

_prog_cache = {}


def _pin_act_tables():
    import concourse.bacc as bacc_mod
    import concourse.mybir as mybir
    if getattr(bacc_mod, "_act_tables_pinned", False):
        return
    orig = bacc_mod.get_activation_tables

    def pinned(arch):
        t = dict(orig(arch))
        keep = "natural_log_exp_and_others"
        drop = {mybir.ActivationFunctionType.Exp,
                mybir.ActivationFunctionType.Ln}
        if keep in t:
            for name in t:
                if name != keep:
                    t[name] = t[name] - drop
        return t

    bacc_mod.get_activation_tables = pinned
    bacc_mod._act_tables_pinned = True


def _build_program(nfull=NFULL, nq=NQ):
    import concourse.bass as bass
    import concourse.tile as tile
    import concourse.mybir as mybir
    from concourse import bacc
    from concourse.masks import make_identity
    _pin_act_tables()

    dt = mybir.dt
    FP32 = dt.float32
    BF16 = dt.bfloat16
    AF = mybir.ActivationFunctionType
    OP = mybir.AluOpType

    NT = nfull // 128      # k/v row tiles
    CT = C // 128          # channel tiles (4)
    QT = nq // 128         # query tiles per core
    KH = max(1, nfull // 1024)   # exp psum chunks (1024 wide)
    KHW = min(1024, nfull)
    Q5 = max(1, nq // 512)       # 512-wide chunks of q
    W5 = min(512, nq)
    NG = 2 if QT >= 2 else 1     # slab groups
    GQT = QT // NG               # q-tiles per group
    GW = GQT * 128               # group width in q

    nc = bacc.Bacc("TRN2", target_bir_lowering=False, debug=False,
                   num_devices=N_CORES)

    xb = nc.dram_tensor("xb", [nfull, C], FP32, kind="ExternalInput").ap()
    maskv = nc.dram_tensor("maskv", [nfull], dt.int32, kind="ExternalInput").ap()
    wqkv = nc.dram_tensor("wqkv", [C, 3 * C], FP32, kind="ExternalInput").ap()
    bqkv = nc.dram_tensor("bqkv", [3 * C], FP32, kind="ExternalInput").ap()
    wproj = nc.dram_tensor("wproj", [C, C], FP32, kind="ExternalInput").ap()
    bproj = nc.dram_tensor("bproj", [C], FP32, kind="ExternalInput").ap()
    out_o = nc.dram_tensor("out_o", [nq, C], FP32, kind="ExternalOutput").ap()
    out_a = nc.dram_tensor("out_a", [nq, nfull], BF16, kind="ExternalOutput").ap()

    with tile.TileContext(nc) as tc:
        from contextlib import ExitStack
        ctx = ExitStack()
        with ctx:
            pers = ctx.enter_context(tc.tile_pool(name="pers", bufs=1))

            # PSUM: S-stream [128,1024] x3 (6 banks) + small [.,<=512] x2
            sp = ctx.enter_context(tc.tile_pool(name="sp", bufs=3, space="PSUM"))
            ovp = ctx.enter_context(tc.tile_pool(name="ovp", bufs=2, space="PSUM"))

            id16 = pers.tile([128, 128], BF16, tag="id16")
            make_identity(nc, id16[:])

            bpb = pers.tile([128, C], FP32, tag="bpb")      # bproj broadcast
            onesrow = pers.tile([1, nfull], BF16, tag="onesrow")
            nc.vector.memset(onesrow[:], 1.0)

            QTM = [pers.tile([D + 1, nq], BF16, tag="qtm", bufs=H,
                             name=f"qtm{i}") for i in range(H)]
            KTM = [pers.tile([D + 1, nfull], BF16, tag="ktm", bufs=H,
                             name=f"ktm{i}") for i in range(H)]
            Vt = pers.tile([128, NT, C], BF16, tag="vt")
            WPt = [pers.tile([D, C], BF16, tag="wp", bufs=H, name=f"wp{i}")
                   for i in range(H)]

            # ---------- setup (transient pools) ----------
            with tc.tile_pool(name="stage", bufs=2) as stg, \
                 tc.tile_pool(name="sxt", bufs=1) as sxt, \
                 tc.tile_pool(name="svec", bufs=1) as svec:

                # mask -> f32 row -> maskbias row (-1e30 on masked keys)
                mri = svec.tile([1, nfull], dt.int32, tag="mri")
                nc.sync.dma_start(mri[:], maskv.unsqueeze(0))
                mrf = svec.tile([1, nfull], FP32, tag="mrf")
                nc.vector.tensor_copy(mrf[:], mri[:])
                mbias = svec.tile([1, nfull], BF16, tag="mbias")
                nc.vector.tensor_scalar(mbias[:], mrf[:], 1e30, -1e30,
                                        op0=OP.mult, op1=OP.add)
                for h in range(H):
                    nc.sync.dma_start(KTM[h][D:D + 1, :], mbias[:])
                    nc.sync.dma_start(QTM[h][D:D + 1, :], onesrow[0:1, 0:nq])

                # b_proj broadcast via ones-column matmul
                bprow = svec.tile([1, C], FP32, tag="bprow")
                nc.sync.dma_start(bprow[:], bproj.unsqueeze(0))
                ones32 = svec.tile([1, 128], FP32, tag="ones32")
                nc.vector.memset(ones32[:], 1.0)
                bb2 = ovp.tile([128, C], FP32, tag="pv", name="bb2")
                nc.tensor.matmul(bb2[:], ones32[:], bprow[:])
                nc.vector.tensor_copy(bpb[:], bb2[:])

                # qkv bias vectors [128, 12] (column t = b[128t:128(t+1)])
                bq12 = svec.tile([128, 3 * CT], FP32, tag="bq12")
                nc.sync.dma_start(bq12[:], bqkv.rearrange("(t p) -> p t", p=128))
                bqs = svec.tile([128, CT], FP32, tag="bqs")  # 0.125 * b_q
                nc.vector.tensor_scalar(bqs[:], bq12[:, 0:CT], SCALE, None,
                                        op0=OP.mult)

                # x -> SBUF bf16, PE-transpose -> xT
                xT = [sxt.tile([128, nfull], BF16, tag="xT", bufs=CT,
                               name=f"xT{i}") for i in range(CT)]
                for nt in range(NT):
                    xs = stg.tile([128, C], FP32, tag="xs")
                    nc.sync.dma_start(xs[:], xb[nt * 128:(nt + 1) * 128, :])
                    xsb = stg.tile([128, C], BF16, tag="xsb")
                    nc.vector.tensor_copy(xsb[:], xs[:])
                    for ct in range(CT):
                        tp = ovp.tile([128, 128], BF16, tag="pv", name="tp")
                        nc.tensor.transpose(tp[:],
                                            xsb[:, ct * 128:(ct + 1) * 128],
                                            id16[:])
                        nc.vector.tensor_copy(
                            xT[ct][:, nt * 128:(nt + 1) * 128], tp[:])

                # W_qkv -> bf16 (Q columns pre-scaled by 0.125)
                Wc = [sxt.tile([128, 3 * C], BF16, tag="wc", bufs=CT,
                               name=f"wc{i}") for i in range(CT)]
                bvrow = sxt.tile([1, C], BF16, tag="bvrow")
                bvf = svec.tile([1, 3 * C], FP32, tag="bvf")
                nc.sync.dma_start(bvf[:], bqkv.unsqueeze(0))
                nc.vector.tensor_copy(bvrow[:], bvf[0:1, 2 * C:3 * C])
                for ct in range(CT):
                    ws = stg.tile([128, 3 * C], FP32, tag="ws")
                    nc.sync.dma_start(ws[:], wqkv[ct * 128:(ct + 1) * 128, :])
                    nc.vector.tensor_scalar(Wc[ct][:, 0:C], ws[:, 0:C], SCALE,
                                            None, op0=OP.mult)
                    nc.vector.tensor_copy(Wc[ct][:, C:3 * C], ws[:, C:3 * C])

                # W_proj -> per-head [64, C] bf16 rows (base partition 0)
                for h in range(H):
                    wps = stg.tile([D, C], FP32, tag="wps")
                    nc.sync.dma_start(wps[:], wproj[h * D:(h + 1) * D, :])
                    nc.vector.tensor_copy(WPt[h][:], wps[:])

                # Q^T d-pair tiles -> per-head QTM rows 0:64 (bias added)
                for j in range(CT):
                    ps = sp.tile([128, nq], FP32, tag="s", name="psq")
                    for ct in range(CT):
                        for q5 in range(Q5):
                            nc.tensor.matmul(
                                ps[:, q5 * W5:(q5 + 1) * W5],
                                Wc[ct][:, j * 128:(j + 1) * 128],
                                xT[ct][:, q5 * W5:(q5 + 1) * W5],
                                start=(ct == 0), stop=(ct == CT - 1))
                    for hh in range(2):
                        nc.vector.tensor_scalar(
                            QTM[2 * j + hh][0:D, :],
                            ps[hh * D:(hh + 1) * D, :],
                            bqs[hh * D:(hh + 1) * D, j:j + 1], None,
                            op0=OP.add)

                # K^T d-pair tiles -> per-head KTM rows 0:64 (bias added)
                for j in range(CT):
                    for kc in range(nfull // nq):
                        ps = sp.tile([128, nq], FP32, tag="s", name="psk")
                        for ct in range(CT):
                            for q5 in range(Q5):
                                o = kc * nq + q5 * W5
                                nc.tensor.matmul(
                                    ps[:, q5 * W5:(q5 + 1) * W5],
                                    Wc[ct][:, C + j * 128:C + (j + 1) * 128],
                                    xT[ct][:, o:o + W5],
                                    start=(ct == 0), stop=(ct == CT - 1))
                        for hh in range(2):
                            nc.vector.tensor_scalar(
                                KTM[2 * j + hh][0:D, kc * nq:(kc + 1) * nq],
                                ps[hh * D:(hh + 1) * D, :],
                                bq12[hh * D:(hh + 1) * D, CT + j:CT + j + 1],
                                None, op0=OP.add)

                # V tiles [128, NT, C] with bias (ones-row trick)
                for nt in range(NT):
                    ps = ovp.tile([128, C], FP32, tag="pv", name="psv")
                    for ct in range(CT):
                        nc.tensor.matmul(ps[:],
                                         xT[ct][:, nt * 128:(nt + 1) * 128],
                                         Wc[ct][:, 2 * C:3 * C],
                                         start=(ct == 0), stop=False)
                    nc.tensor.matmul(ps[:],
                                     onesrow[0:1, nt * 128:(nt + 1) * 128],
                                     bvrow[0:1, :], start=False, stop=True)
                    nc.vector.tensor_copy(Vt[:, nt, :], ps[:])

            # ---------- main-loop pools ----------
            pMAIN = ctx.enter_context(tc.tile_pool(name="pMAIN", bufs=1))
            pSLAB = ctx.enter_context(tc.tile_pool(name="pSLAB", bufs=2))
            pPH = ctx.enter_context(tc.tile_pool(name="pPH", bufs=3))
            pPP = ctx.enter_context(tc.tile_pool(name="pPP", bufs=3))
            pZ = ctx.enter_context(tc.tile_pool(name="pZ", bufs=4))
            pOUT = ctx.enter_context(tc.tile_pool(name="pOUT", bufs=2))

            At = [pMAIN.tile([128, nfull], BF16, tag="a", bufs=QT,
                             name=f"a{i}") for i in range(QT)]
            OTt = [pMAIN.tile([D, nq], BF16, tag="oth", bufs=H,
                              name=f"oth{i}") for i in range(H)]
            RZt = [pMAIN.tile([128, H], FP32, tag="rz", bufs=QT,
                              name=f"rz{i}") for i in range(QT)]

            # ---------- main loop ----------
            ov_state = {}

            def _emit_pv(h, g, slabs, kt0, nkt):
                w = min(512, GW)
                for g5 in range(GW // 512 if GW >= 512 else 1):
                    key = (g, g5)
                    if kt0 == 0:
                        ov_state[key] = ovp.tile([D, w], FP32, tag="pv",
                                                 name="ov")
                    ov = ov_state[key]
                    for kt in range(kt0, kt0 + nkt):
                        nc.tensor.matmul(
                            ov[:],
                            Vt[:, kt, h * D:(h + 1) * D],
                            slabs[g][:, kt, g5 * w:(g5 + 1) * w],
                            start=(kt == 0), stop=(kt == NT - 1))
                    if kt0 + nkt == NT:
                        nc.vector.tensor_copy(
                            OTt[h][:, g * GW + g5 * w:g * GW + (g5 + 1) * w],
                            ov[:])

            for h in range(H):
                slabs = [pSLAB.tile([128, NT, GW], BF16, tag="slab",
                                    name=f"slab{h}_{g}") for g in range(NG)]
                for qt in range(QT):
                    tl = []
                    for kh in range(KH):
                        ps = sp.tile([128, KHW], FP32, tag="s", name="ps")
                        for c5 in range(KHW // 512 if KHW >= 512 else 1):
                            w = min(512, KHW)
                            o = kh * KHW + c5 * w
                            nc.tensor.matmul(
                                ps[:, c5 * w:(c5 + 1) * w],
                                QTM[h][:, qt * 128:(qt + 1) * 128],
                                KTM[h][:, o:o + w])
                        tl.append(ps)
                    # one exp pass; accum_out gives row-sum partials
                    ph = pPH.tile([128, nfull], BF16, tag="ph")
                    zps = []
                    for kh in range(KH):
                        zp = pZ.tile([128, 1], FP32, tag="zp", name="zp")
                        pp = pPP.tile([128, KHW], BF16, tag="pp", name="pp")
                        nc.scalar.activation(pp[:], tl[kh][:], AF.Exp,
                                             accum_out=zp[:])
                        zps.append((zp, pp))
                    if KH == 2:
                        nc.vector.tensor_tensor(zps[0][0][:], zps[0][0][:],
                                                zps[1][0][:], op=OP.add)
                    nc.vector.reciprocal(RZt[qt][:, h:h + 1], zps[0][0][:])
                    # P-hat = P~ * rz  (normalized + masked)
                    for kh in range(KH):
                        nc.vector.tensor_scalar(
                            ph[:, kh * KHW:(kh + 1) * KHW], zps[kh][1][:],
                            RZt[qt][:, h:h + 1], None, op0=OP.mult)
                    # attn-mean accumulation
                    if h == 0:
                        nc.vector.tensor_copy(At[qt][:], ph[:])
                    else:
                        nc.vector.tensor_tensor(At[qt][:], ph[:], At[qt][:],
                                                op=OP.add)
                    # k-major transpose into the slab group
                    g = qt // GQT
                    nc.sync.dma_start_transpose(
                        slabs[g][:, :, (qt % GQT) * 128:(qt % GQT + 1) * 128],
                        ph[:])
                # O^T = V.T @ P-hat^T accumulated over k tiles
                for g in range(NG):
                    _emit_pv(h, g, slabs, 0, NT)

            # ---------- tail: DMA attn out; projection ----------
            for qt in range(QT):
                nc.sync.dma_start(out_a[qt * 128:(qt + 1) * 128, :], At[qt][:])

            for qt in range(QT):
                pj = ovp.tile([128, C], FP32, tag="pv", name="pj")
                for h in range(H):
                    nc.tensor.matmul(pj[:],
                                     OTt[h][:, qt * 128:(qt + 1) * 128],
                                     WPt[h][:], start=(h == 0),
                                     stop=(h == H - 1))
                ob = pOUT.tile([128, C], FP32, tag="ob")
                nc.vector.tensor_tensor(ob[:], pj[:], bpb[:], op=OP.add)
                nc.sync.dma_start(out_o[qt * 128:(qt + 1) * 128, :], ob[:])

    nc.compile()
    return nc


def _get_program(nfull=NFULL, nq=NQ):
    key = (nfull, nq)
    if key not in _prog_cache:
        _prog_cache[key] = _build_program(nfull, nq)
    return _prog_cache[key]


def _run(x, mask, W_qkv, b_qkv, W_proj, b_proj, nfull, nq):
    from concourse.bass_utils import run_bass_kernel_spmd

    nbatch = x.shape[0]
    halves = N_CORES // nbatch
    nc = _get_program(nfull, nq)
    in_maps = []
    for c in range(N_CORES):
        b, j = c // halves, c % halves
        in_maps.append({
            "xb": np.roll(x[b], -j * nq, axis=0) if j else x[b],
            "maskv": np.roll(mask[b], -j * nq) if j else mask[b],
            "wqkv": W_qkv, "bqkv": b_qkv, "wproj": W_proj, "bproj": b_proj,
        })
    res = run_bass_kernel_spmd(nc, in_maps, list(range(N_CORES)))

    out = np.empty((nbatch, nfull, C), np.float32)
    attn = np.empty((nbatch, nfull, nfull), np.float32)
    for c in range(N_CORES):
        b, j = c // halves, c % halves
        out[b, j * nq:(j + 1) * nq] = res.results[c]["out_o"]
        a = res.results[c]["out_a"].astype(np.float32) / H
        if j:
            a = np.roll(a, j * nq, axis=1)
        attn[b, j * nq:(j + 1) * nq] = a
    return out, attn


def kernel(x, mask, W_qkv, b_qkv, W_proj, b_proj):
    return _run(np.asarray(x, np.float32), np.asarray(mask, np.int32),
                np.asarray(W_qkv, np.float32), np.asarray(b_qkv, np.float32),
                np.asarray(W_proj, np.float32), np.asarray(b_proj, np.float32),
                NFULL, NQ)


# revision 23
# speedup vs baseline: 1.0787x; 1.0221x over previous
"""Trainium2 Bass kernel for nn_MultiHeadAttention (B=4, N=2048, C=512, H=8).

Returns (out [B,N,C] f32, attn_mean [B,N,N] f32) like the reference.

Sharding: 8 cores = 4 batches x 2 query-halves, no collectives. The
query-half selection is done by rolling x/mask on the host so the SPMD
program is identical on every core; attn_mean columns are rolled back on
the host.

Per-core dataflow (matmuls bf16, PSUM accumulation f32):
  setup: x^T via PE transpose (bf16); per-head Q~^T [65, nq] (0.125-scaled
         rows + ones row) and K~^T [65, nfull] (rows + maskbias row of
         -1e30 on masked keys); V [k, C] (+bias via ones-row matmul).
  per head, per q-tile: S = Q~^T.T @ K~^T (mask folded in) -> one exp pass
         (ACT accum_out gives the softmax denominator Z for free) ->
         P-hat = P~ * (1/Z)  (DVE 4x) -> attn_mean accumulates P-hat (DVE
         2x adds); P-hat is DMA-xbar-transposed into k-major slabs feeding
         O^T = V.T @ P-hat^T (PE, comes out pre-normalized), and O^T rows
         feed the output projection directly (no PE transposes of O).
"""

import numpy as np
import ml_dtypes

B, NFULL, C = 4, 2048, 512
H, D = 8, 64
NQ = NFULL // 2  # query rows per core
N_CORES = 8
SCALE = D ** -0.5

_prog_cache = {}


def _pin_act_tables():
    import concourse.bacc as bacc_mod
    import concourse.mybir as mybir
    if getattr(bacc_mod, "_act_tables_pinned", False):
        return
    orig = bacc_mod.get_activation_tables

    def pinned(arch):
        t = dict(orig(arch))
        keep = "natural_log_exp_and_others"
        drop = {mybir.ActivationFunctionType.Exp,
                mybir.ActivationFunctionType.Ln}
        if keep in t:
            for name in t:
                if name != keep:
                    t[name] = t[name] - drop
        return t

    bacc_mod.get_activation_tables = pinned
    bacc_mod._act_tables_pinned = True


def _build_program(nfull=NFULL, nq=NQ):
    import concourse.bass as bass
    import concourse.tile as tile
    import concourse.mybir as mybir
    from concourse import bacc
    from concourse.masks import make_identity
    _pin_act_tables()

    dt = mybir.dt
    FP32 = dt.float32
    BF16 = dt.bfloat16
    AF = mybir.ActivationFunctionType
    OP = mybir.AluOpType

    NT = nfull // 128      # k/v row tiles
    CT = C // 128          # channel tiles (4)
    QT = nq // 128         # query tiles per core
    KH = max(1, nfull // 1024)   # exp psum chunks (1024 wide)
    KHW = min(1024, nfull)
    Q5 = max(1, nq // 512)       # 512-wide chunks of q
    W5 = min(512, nq)
    NG = 2 if QT >= 2 else 1     # slab groups
    GQT = QT // NG               # q-tiles per group
    GW = GQT * 128               # group width in q

    nc = bacc.Bacc("TRN2", target_bir_lowering=False, debug=False,
                   num_devices=N_CORES)

    xb = nc.dram_tensor("xb", [nfull, C], FP32, kind="ExternalInput").ap()
    maskv = nc.dram_tensor("maskv", [nfull], dt.int32, kind="ExternalInput").ap()
    wqkv = nc.dram_tensor("wqkv", [C, 3 * C], FP32, kind="ExternalInput").ap()
    bqkv = nc.dram_tensor("bqkv", [3 * C], FP32, kind="ExternalInput").ap()
    wproj = nc.dram_tensor("wproj", [C, C], FP32, kind="ExternalInput").ap()
    bproj = nc.dram_tensor("bproj", [C], FP32, kind="ExternalInput").ap()
    out_o = nc.dram_tensor("out_o", [nq, C], FP32, kind="ExternalOutput").ap()
    out_a = nc.dram_tensor("out_a", [nq, nfull], BF16, kind="ExternalOutput").ap()

    with tile.TileContext(nc) as tc:
        from contextlib import ExitStack
        ctx = ExitStack()
        with ctx:
            pers = ctx.enter_context(tc.tile_pool(name="pers", bufs=1))

            # PSUM: S-stream [128,1024] x3 (6 banks) + small [.,<=512] x2
            sp = ctx.enter_context(tc.tile_pool(name="sp", bufs=3, space="PSUM"))
            ovp = ctx.enter_context(tc.tile_pool(name="ovp", bufs=2, space="PSUM"))

            id16 = pers.tile([128, 128], BF16, tag="id16")
            make_identity(nc, id16[:])

            bpb = pers.tile([128, C], FP32, tag="bpb")      # bproj broadcast
            onesrow = pers.tile([1, nfull], BF16, tag="onesrow")
            nc.vector.memset(onesrow[:], 1.0)

            QTM = [pers.tile([D + 1, nq], BF16, tag="qtm", bufs=H,
                             name=f"qtm{i}") for i in range(H)]
            KTM = [pers.tile([D + 1, nfull], BF16, tag="ktm", bufs=H,
                             name=f"ktm{i}") for i in range(H)]
            Vt = pers.tile([128, NT, C], BF16, tag="vt")
            WPt = [pers.tile([D, C], BF16, tag="wp", bufs=H, name=f"wp{i}")
                   for i in range(H)]

            # ---------- setup (transient pools) ----------
            with tc.tile_pool(name="stage", bufs=2) as stg, \
                 tc.tile_pool(name="sxt", bufs=1) as sxt, \
                 tc.tile_pool(name="svec", bufs=1) as svec:

                # mask -> f32 row -> maskbias row (-1e30 on masked keys)
                mri = svec.tile([1, nfull], dt.int32, tag="mri")
                nc.sync.dma_start(mri[:], maskv.unsqueeze(0))
                mrf = svec.tile([1, nfull], FP32, tag="mrf")
                nc.vector.tensor_copy(mrf[:], mri[:])
                mbias = svec.tile([1, nfull], BF16, tag="mbias")
                nc.vector.tensor_scalar(mbias[:], mrf[:], 1e30, -1e30,
                                        op0=OP.mult, op1=OP.add)
                for h in range(H):
                    nc.sync.dma_start(KTM[h][D:D + 1, :], mbias[:])
                    nc.sync.dma_start(QTM[h][D:D + 1, :], onesrow[0:1, 0:nq])

                # b_proj broadcast via ones-column matmul
                bprow = svec.tile([1, C], FP32, tag="bprow")
                nc.sync.dma_start(bprow[:], bproj.unsqueeze(0))
                ones32 = svec.tile([1, 128], FP32, tag="ones32")
                nc.vector.memset(ones32[:], 1.0)
                bb2 = ovp.tile([128, C], FP32, tag="pv", name="bb2")
                nc.tensor.matmul(bb2[:], ones32[:], bprow[:])
                nc.vector.tensor_copy(bpb[:], bb2[:])

                # qkv bias vectors [128, 12] (column t = b[128t:128(t+1)])
                bq12 = svec.tile([128, 3 * CT], FP32, tag="bq12")
                nc.sync.dma_start(bq12[:], bqkv.rearrange("(t p) -> p t", p=128))
                bqs = svec.tile([128, CT], FP32, tag="bqs")  # 0.125 * b_q
                nc.vector.tensor_scalar(bqs[:], bq12[:, 0:CT], SCALE, None,
                                        op0=OP.mult)

                # x -> SBUF bf16, PE-transpose -> xT
                xT = [sxt.tile([128, nfull], BF16, tag="xT", bufs=CT,
                               name=f"xT{i}") for i in range(CT)]
                for nt in range(NT):
                    xs = stg.tile([128, C], FP32, tag="xs")
                    nc.sync.dma_start(xs[:], xb[nt * 128:(nt + 1) * 128, :])
                    xsb = stg.tile([128, C], BF16, tag="xsb")
                    nc.vector.tensor_copy(xsb[:], xs[:])
                    for ct in range(CT):
                        tp = ovp.tile([128, 128], BF16, tag="pv", name="tp")
                        nc.tensor.transpose(tp[:],
                                            xsb[:, ct * 128:(ct + 1) * 128],
                                            id16[:])
                        nc.vector.tensor_copy(
                            xT[ct][:, nt * 128:(nt + 1) * 128], tp[:])

                # W_qkv -> bf16 (Q columns pre-scaled by 0.125)
                Wc = [sxt.tile([128, 3 * C], BF16, tag="wc", bufs=CT,
                               name=f"wc{i}") for i in range(CT)]
                bvrow = sxt.tile([1, C], BF16, tag="bvrow")
                bvf = svec.tile([1, 3 * C], FP32, tag="bvf")
                nc.sync.dma_start(bvf[:], bqkv.unsqueeze(0))
                nc.vector.tensor_copy(bvrow[:], bvf[0:1, 2 * C:3 * C])
                for ct in range(CT):
                    ws = stg.tile([128, 3 * C], FP32, tag="ws")
                    nc.sync.dma_start(ws[:], wqkv[ct * 128:(ct + 1) * 128, :])
                    nc.vector.tensor_scalar(Wc[ct][:, 0:C], ws[:, 0:C], SCALE,
                                            None, op0=OP.mult)
                    nc.vector.tensor_copy(Wc[ct][:, C:3 * C], ws[:, C:3 * C])

                # W_proj -> per-head [64, C] bf16 rows (base partition 0)
                for h in range(H):
                    wps = stg.tile([D, C], FP32, tag="wps")
                    nc.sync.dma_start(wps[:], wproj[h * D:(h + 1) * D, :])
                    nc.vector.tensor_copy(WPt[h][:], wps[:])

                # Q^T d-pair tiles -> per-head QTM rows 0:64 (bias added)
                for j in range(CT):
                    ps = sp.tile([128, nq], FP32, tag="s", name="psq")
                    for ct in range(CT):
                        for q5 in range(Q5):
                            nc.tensor.matmul(
                                ps[:, q5 * W5:(q5 + 1) * W5],
                                Wc[ct][:, j * 128:(j + 1) * 128],
                                xT[ct][:, q5 * W5:(q5 + 1) * W5],
                                start=(ct == 0), stop=(ct == CT - 1))
                    for hh in range(2):
                        nc.vector.tensor_scalar(
                            QTM[2 * j + hh][0:D, :],
                            ps[hh * D:(hh + 1) * D, :],
                            bqs[hh * D:(hh + 1) * D, j:j + 1], None,
                            op0=OP.add)

                # K^T d-pair tiles -> per-head KTM rows 0:64 (bias added)
                for j in range(CT):
                    for kc in range(nfull // nq):
                        ps = sp.tile([128, nq], FP32, tag="s", name="psk")
                        for ct in range(CT):
                            for q5 in range(Q5):
                                o = kc * nq + q5 * W5
                                nc.tensor.matmul(
                                    ps[:, q5 * W5:(q5 + 1) * W5],
                                    Wc[ct][:, C + j * 128:C + (j + 1) * 128],
                                    xT[ct][:, o:o + W5],
                                    start=(ct == 0), stop=(ct == CT - 1))
                        for hh in range(2):
                            nc.vector.tensor_scalar(
                                KTM[2 * j + hh][0:D, kc * nq:(kc + 1) * nq],
                                ps[hh * D:(hh + 1) * D, :],
                                bq12[hh * D:(hh + 1) * D, CT + j:CT + j + 1],
                                None, op0=OP.add)

                # V tiles [128, NT, C] with bias (ones-row trick)
                for nt in range(NT):
                    ps = ovp.tile([128, C], FP32, tag="pv", name="psv")
                    for ct in range(CT):
                        nc.tensor.matmul(ps[:],
                                         xT[ct][:, nt * 128:(nt + 1) * 128],
                                         Wc[ct][:, 2 * C:3 * C],
                                         start=(ct == 0), stop=False)
                    nc.tensor.matmul(ps[:],
                                     onesrow[0:1, nt * 128:(nt + 1) * 128],
                                     bvrow[0:1, :], start=False, stop=True)
                    nc.vector.tensor_copy(Vt[:, nt, :], ps[:])

            # ---------- main-loop pools ----------
            pMAIN = ctx.enter_context(tc.tile_pool(name="pMAIN", bufs=1))
            pSLAB = ctx.enter_context(tc.tile_pool(name="pSLAB", bufs=3))
            pPH = ctx.enter_context(tc.tile_pool(name="pPH", bufs=3))
            pPP = ctx.enter_context(tc.tile_pool(name="pPP", bufs=3))
            pZ = ctx.enter_context(tc.tile_pool(name="pZ", bufs=4))
            pOUT = ctx.enter_context(tc.tile_pool(name="pOUT", bufs=2))

            At = [pMAIN.tile([128, nfull], BF16, tag="a", bufs=QT,
                             name=f"a{i}") for i in range(QT)]
            OTt = [pMAIN.tile([D, nq], BF16, tag="oth", bufs=H,
                              name=f"oth{i}") for i in range(H)]
            RZt = [pMAIN.tile([128, H], FP32, tag="rz", bufs=QT,
                              name=f"rz{i}") for i in range(QT)]

            # ---------- main loop ----------
            ov_state = {}

            def _emit_pv(h, g, slabs, kt0, nkt):
                w = min(512, GW)
                for g5 in range(GW // 512 if GW >= 512 else 1):
                    key = (g, g5)
                    if kt0 == 0:
                        ov_state[key] = ovp.tile([D, w], FP32, tag="pv",
                                                 name="ov")
                    ov = ov_state[key]
                    for kt in range(kt0, kt0 + nkt):
                        nc.tensor.matmul(
                            ov[:],
                            Vt[:, kt, h * D:(h + 1) * D],
                            slabs[g][:, kt, g5 * w:(g5 + 1) * w],
                            start=(kt == 0), stop=(kt == NT - 1))
                    if kt0 + nkt == NT:
                        nc.vector.tensor_copy(
                            OTt[h][:, g * GW + g5 * w:g * GW + (g5 + 1) * w],
                            ov[:])

            prev = None  # (h, slabs) with group-1 PV still pending
            for h in range(H):
                slabs = [pSLAB.tile([128, NT, GW], BF16, tag="slab",
                                    name=f"slab{h}_{g}") for g in range(NG)]
                for qt in range(QT):
                    tl = []
                    for kh in range(KH):
                        ps = sp.tile([128, KHW], FP32, tag="s", name="ps")
                        for c5 in range(KHW // 512 if KHW >= 512 else 1):
                            w = min(512, KHW)
                            o = kh * KHW + c5 * w
                            nc.tensor.matmul(
                                ps[:, c5 * w:(c5 + 1) * w],
                                QTM[h][:, qt * 128:(qt + 1) * 128],
                                KTM[h][:, o:o + w])
                        tl.append(ps)
                    # one exp pass; accum_out gives row-sum partials
                    ph = pPH.tile([128, nfull], BF16, tag="ph")
                    zps = []
                    for kh in range(KH):
                        zp = pZ.tile([128, 1], FP32, tag="zp", name="zp")
                        pp = pPP.tile([128, KHW], BF16, tag="pp", name="pp")
                        nc.scalar.activation(pp[:], tl[kh][:], AF.Exp,
                                             accum_out=zp[:])
                        zps.append((zp, pp))
                    if KH == 2:
                        nc.vector.tensor_tensor(zps[0][0][:], zps[0][0][:],
                                                zps[1][0][:], op=OP.add)
                    nc.vector.reciprocal(RZt[qt][:, h:h + 1], zps[0][0][:])
                    # P-hat = P~ * rz  (normalized + masked)
                    for kh in range(KH):
                        nc.vector.tensor_scalar(
                            ph[:, kh * KHW:(kh + 1) * KHW], zps[kh][1][:],
                            RZt[qt][:, h:h + 1], None, op0=OP.mult)
                    # attn-mean accumulation
                    if h == 0:
                        nc.vector.tensor_copy(At[qt][:], ph[:])
                    else:
                        nc.vector.tensor_tensor(At[qt][:], ph[:], At[qt][:],
                                                op=OP.add)
                    # k-major transpose into the slab group
                    g = qt // GQT
                    nc.sync.dma_start_transpose(
                        slabs[g][:, :, (qt % GQT) * 128:(qt % GQT + 1) * 128],
                        ph[:])
                    if NG == 2:
                        step = NT // GQT
                        if qt < GQT and prev is not None:
                            _emit_pv(prev[0], 1, prev[1], qt * step, step)
                        elif qt >= GQT:
                            _emit_pv(h, 0, slabs, (qt - GQT) * step, step)
                if NG == 2:
                    prev = (h, slabs)
                else:
                    _emit_pv(h, 0, slabs, 0, NT)
            if NG == 2 and prev is not None:
                _emit_pv(prev[0], 1, prev[1], 0, NT)

            # ---------- tail: DMA attn out; projection ----------
            for qt in range(QT):
                nc.sync.dma_start(out_a[qt * 128:(qt + 1) * 128, :], At[qt][:])

            for qt in range(QT):
                pj = ovp.tile([128, C], FP32, tag="pv", name="pj")
                for h in range(H):
                    nc.tensor.matmul(pj[:],
                                     OTt[h][:, qt * 128:(qt + 1) * 128],
                                     WPt[h][:], start=(h == 0),
                                     stop=(h == H - 1))
                ob = pOUT.tile([128, C], FP32, tag="ob")
                nc.vector.tensor_tensor(ob[:], pj[:], bpb[:], op=OP.add)
                nc.sync.dma_start(out_o[qt * 128:(qt + 1) * 128, :], ob[:])

    nc.compile()
    return nc


def _get_program(nfull=NFULL, nq=NQ):
    key = (nfull, nq)
    if key not in _prog_cache:
        _prog_cache[key] = _build_program(nfull, nq)
    return _prog_cache[key]


def _run(x, mask, W_qkv, b_qkv, W_proj, b_proj, nfull, nq):
    from concourse.bass_utils import run_bass_kernel_spmd

    nbatch = x.shape[0]
    halves = N_CORES // nbatch
    nc = _get_program(nfull, nq)
    in_maps = []
    for c in range(N_CORES):
        b, j = c // halves, c % halves
        in_maps.append({
            "xb": np.roll(x[b], -j * nq, axis=0) if j else x[b],
            "maskv": np.roll(mask[b], -j * nq) if j else mask[b],
            "wqkv": W_qkv, "bqkv": b_qkv, "wproj": W_proj, "bproj": b_proj,
        })
    res = run_bass_kernel_spmd(nc, in_maps, list(range(N_CORES)))

    out = np.empty((nbatch, nfull, C), np.float32)
    attn = np.empty((nbatch, nfull, nfull), np.float32)
    for c in range(N_CORES):
        b, j = c // halves, c % halves
        out[b, j * nq:(j + 1) * nq] = res.results[c]["out_o"]
        a = res.results[c]["out_a"].astype(np.float32) / H
        if j:
            a = np.roll(a, j * nq, axis=1)
        attn[b, j * nq:(j + 1) * nq] = a
    return out, attn


def kernel(x, mask, W_qkv, b_qkv, W_proj, b_proj):
    return _run(np.asarray(x, np.float32), np.asarray(mask, np.int32),
                np.asarray(W_qkv, np.float32), np.asarray(b_qkv, np.float32),
                np.asarray(W_proj, np.float32), np.asarray(b_proj, np.float32),
                NFULL, NQ)
